# revision 20
# baseline (speedup 1.0000x reference)
"""CategoryCrossAttention Trainium2 kernel — 8 NeuronCores, data-parallel over B.

Reference computation (B=32, T=2048, D=1024, C=512, H=1024):
    xn  = LN(x) ; cn = LN(cat_emb)
    q   = cn @ Wq.T ; k,v = xn @ Wk.T, xn @ Wv.T
    wei = softmax(q.k / sqrt(H)) ; out = sum_t wei*v
    proj = out @ Wp.T ; y = x + proj[:,None,:]

Algebraic rewrite (exact): contract the weights with the small side first,
pre-fusing every weight-weight / weight-LN-param product on the host
(input-independent, like torch module init). With zc = plain-normalized
cat_emb (no affine) and M1 = Wq.T @ Wk / sqrt(H):
    kqg_b  = zc_b @ M1g + kq0g          M1g  = diag(c_g) M1 diag(x_g)  [C,D]
    S_b    = zc_b @ M1aux[:,0] + s0     M1aux[:,0] = rowsum(M1g)
    cb_b   = zc_b @ M1aux[:,1] + cb0    M1aux[:,1] = diag(c_g) M1 x_b
    kqgc_b = kqg_b - S_b/D              (centered: the mu term drops out)
    dots[t] = x[t] . kqgc_b ; logits = rstd[t]*dots[t] + cb_b
    w2 = softmax(logits)*rstd ; xw = sum_t w2[t]*x[t]
    zg = x_g*(xw - rowsum(xw)/D) ; proj_b = zg @ M2 + c2 ; y = x + proj
    M2 = (Wp @ Wv).T [D,D] ; c2 = x_b @ M2
This removes all O(B*T*D*H) GEMMs; the kernel is HBM-bound: read x once
(f32), keep it in SBUF as bf16, write y once (bf16).

Engine budget per 128-row tile (64 tiles/core, DMA pace ~2.2us/tile):
  ACT  cast f32->bf16 with accum_out=rowsum(x), 1/4 of dot-accums, half
       the out-DMA issues
  DVE  sumsq (tensor_tensor_reduce), 3/4 of dots (TTR), softmax bits
  Pool y-add (bf16) and prologue weight DMAs (SWDGE); DVE takes the last
       batch's y-adds (tail)
  PE   pass-2 matmuls, proj matmuls, transposes, broadcasts
Sharding: batch-parallel, 4 batches per core, no collectives.
"""
import numpy as np
import ml_dtypes
from contextlib import ExitStack

import concourse.bass as bass
import concourse.tile as tile
from concourse import mybir, masks
from concourse.bass_utils import run_bass_kernel_spmd
from concourse.vector_clock import ScopedClock

B, T, D, C, H = 32, 2048, 1024, 512, 1024
NCORES = 8
BL = B // NCORES          # 4 batches per core
NT = T // 128             # 16 row tiles per batch
ND = D // 128
NCC = C // 128
EPS = 1e-5

F32 = mybir.dt.float32
BF16 = mybir.dt.bfloat16
AX = mybir.AxisListType
OP = mybir.AluOpType
AF = mybir.ActivationFunctionType


# ---------------------------------------------------------------------------
# Walrus in this container encodes at most ONE sem wait per instruction.
# Two workarounds: (1) the Tile kernel-tail drain aggregates many waits ->
# replace with single-wait NOPs; (2) post-pass hoists extra waits from any
# instruction onto single-wait NOPs inserted before it (same engine).
# ---------------------------------------------------------------------------
class PatchedTileContext(tile.TileContext):
    def _drain_and_barrier(self, tick_clock, wait_clock):
        probe = self.nc.sync.nop()
        wait_clock.add_sem_waits(
            probe.ins, ScopedClock({None: tick_clock.global_clock})
        )
        si = probe.ins.sync_info
        waits = list(si.on_wait) if si and si.on_wait else []
        if len(waits) > 1:
            probe.ins.sync_info = mybir.SyncInfo(
                on_wait=waits[:1], on_update=list(si.on_update or [])
            )
            for w in waits[1:]:
                n2 = self.nc.sync.nop()
                n2.ins.sync_info = mybir.SyncInfo(on_wait=[w], on_update=[])
        self.nc.sync.drain()
        self.nc.all_engine_barrier()
        popped = self.nc._tile_sem_poison_stack.pop()
        assert popped is self._sem_poison
        self.nc.clear_and_free_semaphores(list(self.sems.allocated().values()))
        self.nc.all_engine_barrier()


_SEQ = [0]


def split_multi_waits(nc):
    for f in nc.m.functions:
        for bb in f.blocks:
            insts = bb.instructions
            need = False
            for i in insts:
                si = i.sync_info
                if si is not None and si.on_wait and len(si.on_wait) > 1:
                    need = True
                    break
            if not need:
                continue
            new = []
            for inst in insts:
                si = inst.sync_info
                waits = list(si.on_wait) if si is not None and si.on_wait else []
                if len(waits) > 1:
                    for w in waits[:-1]:
                        _SEQ[0] += 1
                        n = mybir.InstNoOp(
                            name=f"waitsplit_{_SEQ[0]}", engine=inst.engine
                        )
                        n.sync_info = mybir.SyncInfo(on_wait=[w], on_update=[])
                        new.append(n)
                    inst.sync_info = mybir.SyncInfo(
                        on_wait=[waits[-1]], on_update=list(si.on_update or [])
                    )
                new.append(inst)
            bb.instructions = new


# ---------------------------------------------------------------------------
# Kernel body
# ---------------------------------------------------------------------------
def _mm(nc, out, lhsT, rhs, start, stop):
    nc.tensor.matmul(out, lhsT, rhs, start=start, stop=stop)


def build_body(ctx, tc, ext):
    nc = tc.nc
    x_ext = ext["x"]
    out_ext = ext["out"]

    # --- persistent pools -------------------------------------------------
    const_p = ctx.enter_context(tc.tile_pool(name="const", bufs=1))
    m2_p = ctx.enter_context(tc.tile_pool(name="m2", bufs=1))
    kqg_p = ctx.enter_context(tc.tile_pool(name="kqg", bufs=1))
    perb_p = ctx.enter_context(tc.tile_pool(name="perb", bufs=2))
    rows_p = ctx.enter_context(tc.tile_pool(name="rows", bufs=1))
    junk_p = ctx.enter_context(tc.tile_pool(name="junk", bufs=1))
    pjbc_p = ctx.enter_context(tc.tile_pool(name="pjbc", bufs=2))
    xbf_p = ctx.enter_context(tc.tile_pool(name="xbf", bufs=40))
    xf_p = ctx.enter_context(tc.tile_pool(name="xf", bufs=6))
    y_p = ctx.enter_context(tc.tile_pool(name="y", bufs=6))
    # PSUM: rowp [<=4,1024] (2 banks) x2 ; tp [<=128,512] (1 bank) x2 ;
    # big [128,1024] (2 banks) x1  => 8 banks total
    ps_row = ctx.enter_context(tc.tile_pool(name="psrow", bufs=2, space="PSUM"))
    ps_tp = ctx.enter_context(tc.tile_pool(name="pstp", bufs=2, space="PSUM"))
    ps_big = ctx.enter_context(tc.tile_pool(name="psbig", bufs=1, space="PSUM"))

    # --- constants (cheap memsets, no DMA) --------------------------------
    ident = const_p.tile([128, 128], F32)
    masks.make_identity(nc, ident[:])
    ident_bf = const_p.tile([128, 128], BF16)
    masks.make_identity(nc, ident_bf[:])
    ones_row = const_p.tile([1, 128], F32)
    nc.vector.memset(ones_row[:], 1.0)
    ones_bf = const_p.tile([1, 128], BF16)
    nc.vector.memset(ones_bf[:], 1.0)
    eps128 = const_p.tile([128, 1], F32)
    nc.vector.memset(eps128[:], EPS)
    eps4 = const_p.tile([BL, 1], F32)
    nc.vector.memset(eps4[:], EPS)
    # aligned e_b selector columns: ecol[:, b, 0] = identity column b
    ecol = const_p.tile([BL, BL, 4], F32)
    nc.vector.memset(ecol[:], 0.0)
    nc.vector.tensor_copy(ecol[:, :, 0], ident[:BL, :BL])

    # --- prologue DMAs ----------------------------------------------------
    # cat on sync (x tiles queue right behind); weights via SWDGE on the
    # otherwise-idle Pool engine so the ACT queue stays clear for casts.
    cat_sb = const_p.tile([BL, C], F32)
    nc.sync.dma_start(cat_sb[:], ext["cat_emb"][:, :])
    m1g = []
    for cc in range(NCC):
        wt = m2_p.tile([128, D], BF16, tag=f"m2{cc}", name=f"m1g{cc}")
        dma_eng = nc.sync if cc % 2 == 0 else nc.scalar
        dma_eng.dma_start(wt[:], ext["M1g"][cc * 128 : (cc + 1) * 128, :])
        m1g.append(wt)
    m1aux = []
    for cc in range(NCC):
        wt = const_p.tile([128, 2], BF16, name=f"m1aux{cc}")
        dma_eng = nc.sync if cc % 2 == 0 else nc.scalar
        dma_eng.dma_start(wt[:], ext["M1aux"][cc * 128 : (cc + 1) * 128, :])
        m1aux.append(wt)
    kq0aug = const_p.tile([1, D + 2], BF16)
    nc.scalar.dma_start(kq0aug[:], ext["kq0aug"][:, :])
    g_row = const_p.tile([1, D], F32)
    c2_row = const_p.tile([1, D], BF16)
    m2t = [None] * ND

    def emit_m2_dma():
        # first needed at proj(b0); emitted after batch 0's x DMAs so the
        # x stream keeps the head of the sync queue.
        nc.scalar.dma_start(g_row[:], ext["ln_x_g"][:, :])
        nc.scalar.dma_start(c2_row[:], ext["c2"][:, :])
        for dc in range(ND):
            wt = m2_p.tile([128, D], BF16, tag=f"m2{dc}", name=f"m2{dc}")
            nc.sync.dma_start(wt[:], ext["M2"][dc * 128 : (dc + 1) * 128, :])
            m2t[dc] = wt

    # --- per-batch persistent tiles --------------------------------------
    kqgc_bc = [
        kqg_p.tile([128, D], BF16, tag=f"kqgc{b}", name=f"kqgc{b}")
        for b in range(BL)
    ]
    cball = kqg_p.tile([128, BL], F32, tag="cball")

    # --- prologue compute --------------------------------------------------
    # cat-LN chain first so its two ACT ops sit at the head of the ACT
    # queue, ahead of the batch-0 casts.
    def emit_prologue_ln(st0_p):
        st6c = st0_p.tile([BL, 6], F32, tag="st6c")
        nc.vector.bn_stats(st6c[:], cat_sb[:])
        stc = st0_p.tile([BL, 2], F32, tag="stc")
        nc.vector.bn_aggr(stc[:], st6c[:])
        sdc = st0_p.tile([BL, 1], F32, tag="sdc")
        nc.scalar.activation(sdc[:], stc[:, 1:2], AF.Ln, bias=eps4[:], scale=1.0)
        rstdc = st0_p.tile([BL, 1], F32, tag="rstdc")
        nc.scalar.activation(rstdc[:], sdc[:], AF.Exp, bias=0.0, scale=-0.5)
        nbc = st0_p.tile([BL, 1], F32, tag="nbc")
        nc.vector.tensor_tensor(out=nbc[:], in0=stc[:, 0:1], in1=rstdc[:], op=OP.mult)
        nc.vector.tensor_scalar(nbc[:], nbc[:], -1.0, None, OP.mult)
        zc = st0_p.tile([BL, C], F32, tag="zc")
        nc.scalar.activation(
            zc[:], cat_sb[:], AF.Identity, bias=nbc[:], scale=rstdc[:]
        )
        return zc

    def emit_prologue_kqg(st0_p, zc):
        # zcT columns [128c x BL] x NCC, bf16 for the M1g matmuls
        zcT_ps = ps_tp.tile([128, NCC * BL], F32, tag="tp")
        for cc in range(NCC):
            nc.tensor.transpose(
                zcT_ps[:, cc * BL : (cc + 1) * BL],
                zc[:, cc * 128 : (cc + 1) * 128],
                ident[:BL, :BL],
            )
        zcT = st0_p.tile([128, NCC * BL], BF16, tag="zcT")
        nc.vector.tensor_copy(zcT[:], zcT_ps[:])
        # kqg = zc @ M1g + kq0g  -> [BL, D]
        kqg_ps = ps_row.tile([BL, D], F32, tag="rowp")
        for cc in range(NCC):
            for df in range(2):
                _mm(
                    nc,
                    kqg_ps[:, df * 512 : (df + 1) * 512],
                    zcT[:, cc * BL : (cc + 1) * BL],
                    m1g[cc][:, df * 512 : (df + 1) * 512],
                    start=(cc == 0),
                    stop=False,
                )
        for df in range(2):
            _mm(
                nc,
                kqg_ps[:, df * 512 : (df + 1) * 512],
                ones_bf[:, :BL],
                kq0aug[:, df * 512 : (df + 1) * 512],
                start=False,
                stop=True,
            )
        # aux = zc @ M1aux + [s0, cb0]  -> [BL, 2] = [S, cb]
        aux_ps = ps_tp.tile([BL, 2], F32, tag="tp")
        for cc in range(NCC):
            _mm(
                nc,
                aux_ps[:],
                zcT[:, cc * BL : (cc + 1) * BL],
                m1aux[cc][:],
                start=(cc == 0),
                stop=False,
            )
        _mm(nc, aux_ps[:], ones_bf[:, :BL], kq0aug[:, D : D + 2], start=False, stop=True)
        aux_sb = st0_p.tile([BL, 2], F32, tag="auxsb")
        nc.vector.tensor_copy(aux_sb[:], aux_ps[:])
        kqg4 = st0_p.tile([BL, D], F32, tag="kqg4")
        nc.vector.tensor_copy(kqg4[:], kqg_ps[:])
        # center: kqgc = kqg - S/D  (makes the mu term vanish)
        S4d = st0_p.tile([BL, 1], F32, tag="S4d")
        nc.vector.tensor_scalar(S4d[:], aux_sb[:, 0:1], 1.0 / D, None, OP.mult)
        kqgc4 = st0_p.tile([BL, D], F32, tag="kqgc4")
        nc.vector.tensor_scalar(kqgc4[:], kqg4[:], S4d[:], None, OP.subtract)
        # broadcast each batch row to [128, D]: PE row-extract (e_b selector)
        # then ones-broadcast, PSUM copies split between ACT and DVE
        for b in range(BL):
            ex_ps = ps_row.tile([1, D], F32, tag="rowp")
            for df in range(2):
                _mm(
                    nc,
                    ex_ps[:, df * 512 : (df + 1) * 512],
                    ecol[:, b, 0:1],
                    kqgc4[:, df * 512 : (df + 1) * 512],
                    start=True,
                    stop=True,
                )
            krow = st0_p.tile([1, D], BF16, tag="krow", name=f"krow{b}")
            nc.vector.tensor_copy(krow[:], ex_ps[:])
            kb_ps = ps_big.tile([128, D], F32, tag="big")
            for df in range(2):
                _mm(
                    nc,
                    kb_ps[:, df * 512 : (df + 1) * 512],
                    ones_bf[:],
                    krow[:, df * 512 : (df + 1) * 512],
                    start=True,
                    stop=True,
                )
            if b % 2 == 0:
                nc.scalar.copy(kqgc_bc[b][:], kb_ps[:])
            else:
                nc.vector.tensor_copy(kqgc_bc[b][:], kb_ps[:])
        # cb broadcast for all batches at once: [4,1] -> [1,4] -> [128,4]
        cbr_ps = ps_tp.tile([1, BL], F32, tag="tp")
        nc.tensor.transpose(cbr_ps[:], aux_sb[:, 1:2], ident[:BL, :BL])
        cbrow = st0_p.tile([1, BL], F32, tag="cbrow")
        nc.vector.tensor_copy(cbrow[:], cbr_ps[:])
        cball_ps = ps_tp.tile([128, BL], F32, tag="tp")
        _mm(nc, cball_ps[:], ones_row[:], cbrow[:], start=True, stop=True)
        nc.vector.tensor_copy(cball[:], cball_ps[:])

    # --- main pipelined loop over batches ---------------------------------
    state = {}

    def emit_pass1_tile(b, ti, st):
        xf = xf_p.tile([128, D], F32, tag="xf")
        nc.sync.dma_start(xf[:], x_ext[b, ti * 128 : (ti + 1) * 128, :])
        xt = xbf_p.tile([128, D], BF16, tag="x")
        nc.scalar.activation(
            xt[:], xf[:], AF.Identity, bias=0.0, scale=1.0,
            accum_out=st["sums"][:, ti : ti + 1],
        )
        st["x_tiles"].append(xt)
        if ti % 2 == 0:
            pj = junk_p.tile([128, D], BF16, tag="sq", bufs=2)
            nc.scalar.activation(
                pj[:], xt[:], AF.Square, bias=0.0, scale=1.0,
                accum_out=st["sumsqs"][:, ti : ti + 1],
            )
        else:
            pj = junk_p.tile([128, D], BF16, tag="sq", bufs=2)
            nc.vector.scalar_tensor_tensor(
                out=pj[:], in0=xt[:], scalar=0.0, in1=xt[:],
                op0=OP.add, op1=OP.mult,
                accum_out=st["sumsqs"][:, ti : ti + 1],
            )

    def emit_dots_tile(b, ti, st):
        xt = st["x_tiles"][ti]
        pj = junk_p.tile([128, D], BF16, tag="dot", bufs=2)
        nc.vector.scalar_tensor_tensor(
            out=pj[:], in0=xt[:], scalar=0.0, in1=kqgc_bc[b][:],
            op0=OP.add, op1=OP.mult,
            accum_out=st["dots"][:, ti : ti + 1],
        )

    def chain_chunks(b, st):
        """softmax -> pass2 -> proj for batch b, split into 7 chunks that are
        emitted one per slot of the NEXT batch's stream so they interleave
        with its casts/dots in every engine queue instead of blocking them.
        The softmax skips the max-subtraction (logits are O(1) here) and
        defers the 1/sum normalization into the proj row, which shortens the
        serial chain considerably."""

        def c0():  # rstd + logits; rstd = exp(-ln(var+eps)/2) keeps all ACT
            # work in one table set (no sqrt <-> exp table switching)
            dots = st["dots"]
            musq = perb_p.tile([128, NT], F32, tag="musq")
            nc.vector.scalar_tensor_tensor(
                out=musq[:], in0=st["sums"][:], scalar=1.0 / (D * D),
                in1=st["sums"][:], op0=OP.mult, op1=OP.mult,
            )
            var = perb_p.tile([128, NT], F32, tag="var")
            nc.vector.scalar_tensor_tensor(
                out=var[:], in0=st["sumsqs"][:], scalar=1.0 / D, in1=musq[:],
                op0=OP.mult, op1=OP.subtract,
            )
            lnv = perb_p.tile([128, NT], F32, tag="lnv")
            nc.scalar.activation(lnv[:], var[:], AF.Ln, bias=eps128[:], scale=1.0)
            rstd = perb_p.tile([128, NT], F32, tag="rstd")
            nc.scalar.activation(rstd[:], lnv[:], AF.Exp, bias=0.0, scale=-0.5)
            st["rstd"] = rstd
            lg = perb_p.tile([128, NT], F32, tag="lg")
            nc.vector.tensor_tensor(out=lg[:], in0=dots[:], in1=rstd[:], op=OP.mult)
            st["lg"] = lg

        def c1():  # exp (cb folded into the bias; no max-sub needed)
            e_t = perb_p.tile([128, NT], F32, tag="e")
            rs = perb_p.tile([128, 1], F32, tag="rs")
            nc.scalar.activation(
                e_t[:], st["lg"][:], AF.Exp, bias=cball[:, b : b + 1], scale=1.0,
                accum_out=rs[:],
            )
            st["e_t"], st["rs"] = e_t, rs

        def c2():  # 1/sum (normalization deferred to proj) + w2
            rst_ps = ps_tp.tile([1, 128], F32, tag="tp")
            nc.tensor.transpose(rst_ps[:], st["rs"][:], ident[:])
            rsr = perb_p.tile([1, 128], F32, tag="rsr")
            nc.scalar.copy(rsr[:], rst_ps[:])
            tot = perb_p.tile([1, 1], F32, tag="tot")
            nc.vector.tensor_reduce(tot[:], rsr[:], axis=AX.X, op=OP.add)
            inv = perb_p.tile([1, 1], F32, tag="inv")
            nc.vector.reciprocal(inv[:], tot[:])
            st["inv"] = inv
            w2_bf = perb_p.tile([128, NT], BF16, tag="w2bf")
            nc.vector.tensor_tensor(
                out=w2_bf[:], in0=st["e_t"][:], in1=st["rstd"][:], op=OP.mult
            )
            st["w2_bf"] = w2_bf

        def c3():  # pass2 first half (unnormalized xw)
            st["xw_ps"] = ps_row.tile([1, D], F32, tag="rowp", name="xwps")
            for ti in range(NT // 2):
                for df in range(2):
                    nc.tensor.matmul(
                        st["xw_ps"][:, df * 512 : (df + 1) * 512],
                        st["w2_bf"][:, ti : ti + 1],
                        st["x_tiles"][ti][:, df * 512 : (df + 1) * 512],
                        start=(ti == 0),
                        stop=False,
                    )

        def c4():  # pass2 second half
            for ti in range(NT // 2, NT):
                for df in range(2):
                    nc.tensor.matmul(
                        st["xw_ps"][:, df * 512 : (df + 1) * 512],
                        st["w2_bf"][:, ti : ti + 1],
                        st["x_tiles"][ti][:, df * 512 : (df + 1) * 512],
                        start=False,
                        stop=(ti == NT - 1),
                    )

        def c5():  # zg = g*(xw - rowsum(xw)/D), transposed to columns
            xw_sb = rows_p.tile([1, D], F32, tag="xwsb")
            s2r = rows_p.tile([1, 1], F32, tag="s2r")
            nc.scalar.activation(
                xw_sb[:], st["xw_ps"][:], AF.Identity, bias=0.0, scale=1.0,
                accum_out=s2r[:],
            )
            s2m = rows_p.tile([1, 1], F32, tag="s2m")
            nc.scalar.mul(s2m[:], s2r[:], 1.0 / D)
            zg_bf = rows_p.tile([1, D], BF16, tag="zgbf")
            nc.vector.scalar_tensor_tensor(
                out=zg_bf[:], in0=xw_sb[:], scalar=s2m[:], in1=g_row[:],
                op0=OP.subtract, op1=OP.mult,
            )
            zgc_ps = ps_tp.tile([128, ND, 8], BF16, tag="tp")
            for dc in range(ND):
                nc.tensor.transpose(
                    zgc_ps[:, dc, 0:1],
                    zg_bf[:, dc * 128 : (dc + 1) * 128],
                    ident_bf[:1, :1],
                )
            zg_cols = rows_p.tile([128, ND], BF16, tag="zgcols")
            nc.scalar.copy(zg_cols[:], zgc_ps[:, :, 0])
            st["zg_cols"] = zg_cols

        def c6():  # proj = (zg @ M2)/tot (+ c2), broadcast to [128, D] bf16
            proj_ps = ps_row.tile([1, D], F32, tag="rowp")
            for dc in range(ND):
                for df in range(2):
                    nc.tensor.matmul(
                        proj_ps[:, df * 512 : (df + 1) * 512],
                        st["zg_cols"][:, dc : dc + 1],
                        m2t[dc][:, df * 512 : (df + 1) * 512],
                        start=(dc == 0),
                        stop=(dc == ND - 1),
                    )
            proj_sb = rows_p.tile([1, D], BF16, tag="projsb")
            nc.vector.tensor_scalar(
                proj_sb[:], proj_ps[:], st["inv"][:], None, OP.mult
            )
            pb_ps = ps_big.tile([128, D], F32, tag="big")
            for df in range(2):
                _mm(
                    nc,
                    pb_ps[:, df * 512 : (df + 1) * 512],
                    ones_bf[:],
                    proj_sb[:, df * 512 : (df + 1) * 512],
                    start=True,
                    stop=False,
                )
                _mm(
                    nc,
                    pb_ps[:, df * 512 : (df + 1) * 512],
                    ones_bf[:],
                    c2_row[:, df * 512 : (df + 1) * 512],
                    start=False,
                    stop=True,
                )
            proj_bc = pjbc_p.tile([128, D], BF16, tag="pjbc")
            nc.scalar.copy(proj_bc[:], pb_ps[:])
            st["proj_bc"] = proj_bc

        return [c0, c1, c2, c3, c4, c5, c6]

    def emit_y_tile(b, ti, st, eng, dma_eng=None):
        yt = y_p.tile([128, D], BF16, tag="y")
        eng.tensor_tensor(
            out=yt[:], in0=st["x_tiles"][ti][:], in1=st["proj_bc"][:], op=OP.add
        )
        (dma_eng or nc.sync).dma_start(
            out_ext[b, ti * 128 : (ti + 1) * 128, :], yt[:]
        )

    def emit_pe_warmup():
        # a dummy transpose keeps the PE HAM window hot ahead of pass2
        wu = ps_tp.tile([128, 128], F32, tag="tp")
        nc.tensor.transpose(wu[:], ident[:], ident[:])

    st0_stack = ExitStack()
    st0_p = st0_stack.enter_context(tc.tile_pool(name="st0", bufs=1))
    zc = emit_prologue_ln(st0_p)

    for b in range(BL):
        st = state[b] = {
            "x_tiles": [],
            "sums": perb_p.tile([128, NT], F32, tag="sums", name=f"sums{b}"),
            "sumsqs": perb_p.tile([128, NT], F32, tag="sumsqs", name=f"sumsqs{b}"),
            "dots": perb_p.tile([128, NT], F32, tag="dots", name=f"dots{b}"),
        }
        prev = state.get(b - 1)
        prev2 = state.get(b - 2)
        chunks = chain_chunks(b - 1, prev) if prev is not None else []
        for ti in range(NT):
            emit_pass1_tile(b, ti, st)
            if b == 0 and ti == 1:
                emit_prologue_kqg(st0_p, zc)
            lag = 4 if b == 0 else 1
            if ti >= lag:
                emit_dots_tile(b, ti - lag, st)
            if prev is not None and ti < len(chunks):
                chunks[ti]()
            if prev2 is not None and ti < 8:
                # second half of b-2's y-adds (its proj landed mid-(b-1))
                emit_y_tile(b - 2, ti + 8, prev2, nc.gpsimd)
            if prev is not None and ti >= 8:
                emit_y_tile(b - 1, ti - 8, prev, nc.gpsimd)
            if ti >= 13:
                emit_pe_warmup()
        for ti in range(NT - (4 if b == 0 else 1), NT):
            emit_dots_tile(b, ti, st)
        if b == 0:
            emit_m2_dma()
            st0_stack.close()
        if prev2 is not None:
            prev2["x_tiles"] = []  # fully consumed now
    # tail: finish b2's y-adds, run b3's chain, then its y-adds split
    # across Pool and DVE (both idle in the tail)
    st2, st3 = state[BL - 2], state[BL - 1]
    for c in chain_chunks(BL - 1, st3):
        c()
    for ti in range(8, NT):
        eng = nc.gpsimd if ti % 2 == 0 else nc.vector
        dma_eng = nc.sync if ti % 2 == 0 else nc.scalar
        emit_y_tile(BL - 2, ti, st2, eng, dma_eng)
    st2["x_tiles"] = []
    for ti in range(NT):
        eng = nc.gpsimd if ti % 2 == 0 else nc.vector
        dma_eng = nc.sync if ti % 2 == 0 else nc.scalar
        emit_y_tile(BL - 1, ti, st3, eng, dma_eng)
    st3["x_tiles"] = []


def build_nc(loop_n: int = 1, split_waits: bool = True):
    nc = bass.Bass("TRN2", target_bir_lowering=False, debug=False)
    ext = {
        "x": nc.declare_dram_parameter("x", [BL, T, D], F32, isOutput=False).ap(),
        "cat_emb": nc.declare_dram_parameter("cat_emb", [BL, C], F32, isOutput=False).ap(),
        "ln_x_g": nc.declare_dram_parameter("ln_x_g", [1, D], F32, isOutput=False).ap(),
        "M1g": nc.declare_dram_parameter("M1g", [C, D], BF16, isOutput=False).ap(),
        "M1aux": nc.declare_dram_parameter("M1aux", [C, 2], BF16, isOutput=False).ap(),
        "kq0aug": nc.declare_dram_parameter("kq0aug", [1, D + 2], BF16, isOutput=False).ap(),
        "M2": nc.declare_dram_parameter("M2", [D, D], BF16, isOutput=False).ap(),
        "c2": nc.declare_dram_parameter("c2", [1, D], BF16, isOutput=False).ap(),
        "out": nc.declare_dram_parameter("out", [BL, T, D], BF16, isOutput=True).ap(),
    }
    with PatchedTileContext(nc) as tc:
        with ExitStack() as ctx:
            if loop_n > 1:
                with tc.For_i(0, loop_n, 1):
                    build_body(ctx, tc, ext)
            else:
                build_body(ctx, tc, ext)
    if split_waits:
        split_multi_waits(nc)
    return nc


_NC_CACHE = {}


def _get_nc(loop_n=1):
    if loop_n not in _NC_CACHE:
        _NC_CACHE[loop_n] = build_nc(loop_n)
    return _NC_CACHE[loop_n]


def make_in_maps(inputs):
    x = np.ascontiguousarray(inputs["x"], dtype=np.float32)
    cat = np.ascontiguousarray(inputs["cat_emb"], dtype=np.float32)
    Wq = np.asarray(inputs["Wq"], np.float32)
    Wk = np.asarray(inputs["Wk"], np.float32)
    Wv = np.asarray(inputs["Wv"], np.float32)
    Wp = np.asarray(inputs["Wp"], np.float32)
    xg = np.asarray(inputs["ln_x_g"], np.float32).reshape(D)
    xb = np.asarray(inputs["ln_x_b"], np.float32).reshape(D)
    cg = np.asarray(inputs["ln_c_g"], np.float32).reshape(C)
    cbv = np.asarray(inputs["ln_c_b"], np.float32).reshape(C)
    M1 = (Wq.T @ Wk) * (float(H) ** -0.5)          # [C, D]
    M1c = cg[:, None] * M1                          # diag(c_g) @ M1
    M1g = M1c * xg[None, :]                         # ... @ diag(x_g)
    kq0 = cbv @ M1                                  # [D]
    kq0g = kq0 * xg
    M1aux = np.stack([M1g.sum(axis=1), M1c @ xb], axis=1)   # [C, 2]
    kq0aug = np.concatenate([kq0g, [kq0g.sum()], [kq0 @ xb]]).reshape(1, D + 2)
    M2 = np.ascontiguousarray((Wp @ Wv).T)         # [D, D]
    c2 = xb.reshape(1, D) @ M2
    shared = {
        "ln_x_g": np.ascontiguousarray(xg, np.float32).reshape(1, D),
        "M1g": np.ascontiguousarray(M1g.astype(ml_dtypes.bfloat16)),
        "M1aux": np.ascontiguousarray(M1aux.astype(ml_dtypes.bfloat16)),
        "kq0aug": np.ascontiguousarray(kq0aug.astype(ml_dtypes.bfloat16)),
        "M2": np.ascontiguousarray(M2.astype(ml_dtypes.bfloat16)),
        "c2": np.ascontiguousarray(c2.astype(ml_dtypes.bfloat16)).reshape(1, D),
    }
    in_maps = []
    for i in range(NCORES):
        m = dict(shared)
        m["x"] = x[i * BL : (i + 1) * BL]
        m["cat_emb"] = cat[i * BL : (i + 1) * BL]
        in_maps.append(m)
    return in_maps


def kernel(**inputs) -> np.ndarray:
    nc = _get_nc(1)
    in_maps = make_in_maps(inputs)
    res = run_bass_kernel_spmd(nc, in_maps, core_ids=list(range(NCORES)))
    out = np.concatenate([res.results[i]["out"] for i in range(NCORES)], axis=0)
    return out.astype(np.float32)
